# revision 22
# baseline (speedup 1.0000x reference)
"""KNN retrieval kernel for Trainium2 (8 NeuronCores, SPMD).

Pipeline (per core, datastore sharded N=500000 -> 8 x 62500):
  host:   shard + transpose features to [128, 4, N_loc] fp8-e4m3 (contraction
          subtile layout for DoubleRow matmuls), queries to [128, 4, 256] fp8.
  device: sims = q @ f in fp8 DoubleRow mode (K=256/pass, fp32 PSUM).
          Scalar engine (or DVE directly from PSUM) extracts each [128, 2x512]
          PSUM unit; DVE folds each 512-col chunk 16x by a pairwise-max tree
          (bf16, 2x perf mode) -> window-max map [128, 124*32] per query block
          (window = 16 cols with stride 32 inside its chunk).
          Per region of <=32 chunks: add iota*2^-13 (unique tie-break) ->
          fp32 map', vector.max top-8 + vector.max_index -> 8 window ids.
  host:   per query 256 candidate windows (8 cores x 32), keep top-160 by
          map value, expand to 16 cols each, exact fp32 rescore with true
          L2 normalization, top-32 with reference tie-break, W = exp(s/T),
          one-hot label aggregation.

Recall: a true top-32 neighbor is missed only if its window-max loses its
region's top-8 (margin ~5 sigma of fp8 noise) or the host top-160 filter
(~5 sigma). Window members are all rescored, so in-window collisions are
harmless.
"""

import numpy as np
import ml_dtypes

import concourse.bass as bass
import concourse.mybir as mybir
from concourse import bacc
from concourse.tile import TileContext
from concourse import bass_utils

P = 128
D = 512                 # feature dim = 4 K-subtiles of 128
NQ = 256                # queries (2 partition blocks)
QB = NQ // P            # 2
NCORES = 8
N_TOTAL = 500000
N_SHARD = N_TOTAL // NCORES    # 62500
CB = 1024               # columns per block (= 2 PSUM banks per query block)
NBLK = (N_SHARD + CB - 1) // CB     # 62
N_PAD = CB * NBLK                   # 63488
CHUNK = 512
CHUNKS = N_PAD // CHUNK             # 124
WMAP = 32               # map entries per chunk (window = 16 cols, stride 32)
WIN = CHUNK // WMAP     # 16
MAPN = CHUNKS * WMAP                # 3968 map entries per query block
REG_CHUNKS = 32         # chunks per top-8 region (<= 1024 map entries)
NREG = (CHUNKS + REG_CHUNKS - 1) // REG_CHUNKS   # 4
EPS_TIE = 2.0 ** -13

K = 32
TEMP = 0.07
NUM_CLASSES = 1000
EPS = 1e-12
W_SEL = 160             # windows kept per query by the host value prefilter

FP8 = mybir.dt.float8e4
NP_FP8 = mybir.dt.np(FP8)

# unit u = 2*block + qb; 'A': scalar-engine activation copy PSUM->SBUF bf16,
# 'V': DVE tensor_copy extract (gpsimd cannot read PSUM on TRN2).
# DVE runs the fold tree for all units.
def _sched(nunits):
    return ['A' for _ in range(nunits)]


_NC_CACHE = None


def build_nc(nblk=NBLK):
    n_pad = nblk * CB
    chunks = n_pad // CHUNK
    nreg = (chunks + REG_CHUNKS - 1) // REG_CHUNKS
    sched = _sched(nblk * QB)

    nc = bacc.Bacc("TRN2", target_bir_lowering=False, debug=False)
    q_dram = nc.dram_tensor("qT8", [P, 4, NQ], FP8, kind="ExternalInput").ap()
    f_dram = nc.dram_tensor("fT8", [P, 4, n_pad], FP8,
                            kind="ExternalInput").ap()
    iota_dram = nc.dram_tensor("iota", [P, REG_CHUNKS * WMAP],
                               mybir.dt.float32, kind="ExternalInput").ap()
    vals_dram = nc.dram_tensor("cand_vals", [QB, P, nreg * 8],
                               mybir.dt.float32, kind="ExternalOutput").ap()

    MAX = mybir.AluOpType.max

    with TileContext(nc) as tc:
        with (
            tc.tile_pool(name="qpool", bufs=1) as qpool,
            tc.tile_pool(name="fpool", bufs=4) as fpool,
            tc.tile_pool(name="spool", bufs=6) as spool,
            tc.tile_pool(name="tpool", bufs=4) as tpool,
            tc.tile_pool(name="mpool", bufs=1) as mpool,
            tc.tile_pool(name="rpool", bufs=2) as rpool,
            tc.tile_pool(name="psum", bufs=2, space="PSUM") as psum_pool,
        ):
            qt = qpool.tile([P, 4, NQ], FP8)
            nc.sync.dma_start(qt, q_dram)
            iota = qpool.tile([P, REG_CHUNKS * WMAP], mybir.dt.float32)

            maps = [mpool.tile([P, chunks, WMAP], mybir.dt.bfloat16,
                               name=f"map{qb}") for qb in range(QB)]
            vout = [mpool.tile([P, nreg * 8], mybir.dt.float32,
                               name=f"vout{qb}") for qb in range(QB)]

            for b in range(nblk):
                ft = fpool.tile([P, 4, CB], FP8, tag="ft")
                nc.sync.dma_start(ft, f_dram[:, :, b * CB:(b + 1) * CB])
                if b == 0:
                    # iota is only needed by the first region reduction
                    # (block 15); keep it off the critical startup path
                    nc.sync.dma_start(iota, iota_dram)

                # one 4-bank PSUM tile per block, qb-major: [qb, c, :]
                pt = psum_pool.tile([P, 4, CHUNK], mybir.dt.float32,
                                    name=f"pt_{b}", tag="pt")
                # weight reuse: same lhsT serves both 512-col chunks
                for kc in range(2):
                    for qb in range(QB):
                        lhsT = qt[:, 2 * kc:2 * kc + 2, qb * P:(qb + 1) * P]
                        for c in range(2):
                            nc.tensor.matmul(
                                pt[:, 2 * qb + c, :],
                                lhsT=lhsT,
                                rhs=ft[:, 2 * kc:2 * kc + 2,
                                       c * CHUNK:(c + 1) * CHUNK],
                                start=(kc == 0), stop=(kc == 1),
                                perf_mode=mybir.MatmulPerfMode.DoubleRow)

                # extract all 4 chunks; on every 6th block DVE casts the qb1
                # half in parallel with ACT (sheds scalar-engine load and
                # releases PSUM sooner)
                s = spool.tile([P, 4, CHUNK], mybir.dt.bfloat16, tag="s")
                if b % 6 == 3:
                    nc.scalar.activation(
                        s[:, 0:2, :], pt[:, 0:2, :],
                        func=mybir.ActivationFunctionType.Copy)
                    nc.vector.tensor_copy(out=s[:, 2:4, :], in_=pt[:, 2:4, :])
                else:
                    nc.scalar.activation(
                        s, pt, func=mybir.ActivationFunctionType.Copy)
                t1 = tpool.tile([P, 4, 256], mybir.dt.bfloat16, tag="t1")
                nc.vector.tensor_tensor(
                    out=t1, in0=s[:, :, 0:256], in1=s[:, :, 256:512], op=MAX)
                t2 = tpool.tile([P, 4, 128], mybir.dt.bfloat16, tag="t2")
                nc.vector.tensor_tensor(
                    out=t2, in0=t1[:, :, 0:128], in1=t1[:, :, 128:256], op=MAX)
                t3 = tpool.tile([P, 4, 64], mybir.dt.bfloat16, tag="t3")
                nc.vector.tensor_tensor(
                    out=t3, in0=t2[:, :, 0:64], in1=t2[:, :, 64:128], op=MAX)
                for qb in range(QB):
                    nc.vector.tensor_tensor(
                        out=maps[qb][:, 2 * b:2 * b + 2, :],
                        in0=t3[:, 2 * qb:2 * qb + 2, 0:32],
                        in1=t3[:, 2 * qb:2 * qb + 2, 32:64], op=MAX)

                # emit region reductions as soon as their chunks are done
                for qb in range(QB):
                    for r in range(nreg):
                        lo = r * REG_CHUNKS
                        hi = min(lo + REG_CHUNKS, chunks)
                        if hi != 2 * (b + 1):
                            continue
                        cnt = (hi - lo) * WMAP
                        mp = rpool.tile([P, REG_CHUNKS * WMAP],
                                        mybir.dt.float32, tag="mp")
                        nc.vector.scalar_tensor_tensor(
                            out=mp[:, 0:cnt], in0=maps[qb][:, lo:hi, :],
                            scalar=1.0, in1=iota[:, 0:cnt],
                            op0=mybir.AluOpType.mult,
                            op1=mybir.AluOpType.add)
                        v8 = vout[qb][:, r * 8:(r + 1) * 8]
                        nc.vector.max(out=v8, in_=mp[:, 0:cnt])

            for qb in range(QB):
                nc.sync.dma_start(vals_dram[qb], vout[qb])
    nc.compile()
    return nc


def _get_nc():
    global _NC_CACHE
    if _NC_CACHE is None:
        _NC_CACHE = build_nc()
    return _NC_CACHE


def _prep_in_maps(queries, f):
    # queries -> [128, 4, 256] fp8: q8[p, s, m] = q[m, 128*s + p]
    qT = np.ascontiguousarray(queries.T)                     # [512, 256]
    q8 = np.ascontiguousarray(
        qT.reshape(4, P, NQ).transpose(1, 0, 2)).astype(NP_FP8)
    iota = np.broadcast_to(
        (np.arange(REG_CHUNKS * WMAP, dtype=np.float32) *
         np.float32(EPS_TIE))[None, :], (P, REG_CHUNKS * WMAP))
    iota = np.ascontiguousarray(iota)

    f8_full = f.astype(NP_FP8)                               # [500000, 512]
    in_maps = []
    for c in range(NCORES):
        shard = f8_full[c * N_SHARD:(c + 1) * N_SHARD]       # [62500, 512]
        fT = shard.T.reshape(4, P, N_SHARD).transpose(1, 0, 2)  # [128,4,N]
        f8 = np.zeros((P, 4, N_PAD), dtype=NP_FP8)
        f8[:, :, :N_SHARD] = fT
        in_maps.append({"qT8": q8, "fT8": f8, "iota": iota})
    return in_maps


def run_device(queries, f, trace=False):
    """Returns (wval, widx, res): per-core window values / global window ids.

    wval/widx: [NCORES, NQ, NREG*8]  (window id = map index in [0, MAPN)).
    The region-local index is decoded from the value's iota*2^-13 payload
    (exact whenever |value| >= 32; smaller values are never competitive)."""
    in_maps = _prep_in_maps(queries, f)
    nc = _get_nc()
    res = bass_utils.run_bass_kernel_spmd(
        nc, in_maps, core_ids=list(range(NCORES)), trace=trace)
    wval = np.stack([np.asarray(res.results[c]["cand_vals"],
                                dtype=np.float32).reshape(NQ, NREG * 8)
                     for c in range(NCORES)])
    b16 = wval.astype(ml_dtypes.bfloat16)
    base = b16.astype(np.float32)
    b_dn = np.nextafter(b16, ml_dtypes.bfloat16(-3e38)).astype(np.float32)
    base = np.where(base > wval, b_dn, base)
    pos = np.rint((wval - base) * np.float32(1.0 / EPS_TIE)).astype(np.int64)
    np.clip(pos, 0, REG_CHUNKS * WMAP - 1, out=pos)
    reg_of = np.arange(NREG * 8) // 8                        # [NREG*8]
    widx = reg_of[None, None, :] * (REG_CHUNKS * WMAP) + pos
    np.clip(widx, 0, MAPN - 1, out=widx)
    return wval, widx, res


def knn_from_windows(queries, f, labels, wval, widx):
    nq = queries.shape[0]
    nwin = NCORES * NREG * 8                                 # 256
    # flatten to [nq, nwin] with core-global window ids
    val = wval.transpose(1, 0, 2).reshape(nq, nwin)
    gwin = (widx + (np.arange(NCORES) * MAPN)[:, None, None]) \
        .transpose(1, 0, 2).reshape(nq, nwin)

    w = min(W_SEL, nwin)
    part = np.argpartition(-val, w - 1, axis=1)[:, :w]
    sel = np.take_along_axis(gwin, part, axis=1)             # [nq, w]

    # expand windows to columns: window j (within core) = chunk j>>5,
    # residue j&31 -> cols chunk*512 + residue + 32k, k=0..15
    core = sel // MAPN
    j = sel % MAPN
    ch = j >> 5
    rs = j & 31
    cols = (ch[:, :, None] * CHUNK + rs[:, :, None]
            + 32 * np.arange(WIN)[None, None, :])            # [nq, w, 16]
    rows = core[:, :, None] * N_SHARD + cols
    valid = cols < N_SHARD
    rows = np.where(valid, rows, 0).reshape(nq, -1)          # [nq, w*16]
    valid = valid.reshape(nq, -1)

    # exact fp32 rescore with the reference normalization
    qn = queries.astype(np.float32)
    qn /= np.maximum(np.linalg.norm(qn, axis=1, keepdims=True), EPS)
    rows_f = f[rows.reshape(-1)].reshape(nq, rows.shape[1], D)
    rows_f = rows_f / np.maximum(
        np.linalg.norm(rows_f, axis=2, keepdims=True), EPS)
    sims = np.einsum('qtd,qd->qt', rows_f, qn, dtype=np.float32)
    sims = np.where(valid, sims, -np.inf)

    # top-32, ties broken by lower row index (jax.lax.top_k semantics)
    by_idx = np.argsort(rows, axis=1, kind='stable')
    sims_s = np.take_along_axis(sims, by_idx, axis=1)
    rows_s = np.take_along_axis(rows, by_idx, axis=1)
    order = np.argsort(-sims_s, axis=1, kind='stable')[:, :K]
    top_sims = np.take_along_axis(sims_s, order, axis=1)
    top_idx = np.take_along_axis(rows_s, order, axis=1)

    wts = np.exp(top_sims.astype(np.float32) / np.float32(TEMP))
    lab = labels[top_idx]
    out = np.zeros((nq, NUM_CLASSES), dtype=np.float32)
    np.add.at(out, (np.arange(nq)[:, None], lab), wts)
    return out


def kernel(queries, train_features, train_labels):
    queries = np.asarray(queries, dtype=np.float32)
    f = np.asarray(train_features, dtype=np.float32)
    labels = np.asarray(train_labels)
    wval, widx, _ = run_device(queries, f)
    return knn_from_windows(queries, f, labels, wval, widx)


# revision 23
# speedup vs baseline: 1.0905x; 1.0905x over previous
"""KNN retrieval kernel for Trainium2 (8 NeuronCores, SPMD).

Pipeline (per core, datastore sharded N=500000 -> 8 x 62500):
  host:   shard + transpose features to [128, 4, N_loc] fp8-e4m3 (contraction
          subtile layout for DoubleRow matmuls), queries to [128, 4, 256] fp8.
  device: sims = q @ f in fp8 DoubleRow mode (K=256/pass, fp32 PSUM).
          Scalar engine (or DVE directly from PSUM) extracts each [128, 2x512]
          PSUM unit; DVE folds each 512-col chunk 16x by a pairwise-max tree
          (bf16, 2x perf mode) -> window-max map [128, 124*32] per query block
          (window = 16 cols with stride 32 inside its chunk).
          Per region of <=32 chunks: add iota*2^-13 (unique tie-break) ->
          fp32 map', vector.max top-8 + vector.max_index -> 8 window ids.
  host:   per query 256 candidate windows (8 cores x 32), keep top-160 by
          map value, expand to 16 cols each, exact fp32 rescore with true
          L2 normalization, top-32 with reference tie-break, W = exp(s/T),
          one-hot label aggregation.

Recall: a true top-32 neighbor is missed only if its window-max loses its
region's top-8 (margin ~5 sigma of fp8 noise) or the host top-160 filter
(~5 sigma). Window members are all rescored, so in-window collisions are
harmless.
"""

import numpy as np
import ml_dtypes

import concourse.bass as bass
import concourse.mybir as mybir
from concourse import bacc
from concourse.tile import TileContext
from concourse import bass_utils

P = 128
D = 512                 # feature dim = 4 K-subtiles of 128
NQ = 256                # queries (2 partition blocks)
QB = NQ // P            # 2
NCORES = 8
N_TOTAL = 500000
N_SHARD = N_TOTAL // NCORES    # 62500
CB = 1024               # columns per block (= 2 PSUM banks per query block)
NBLK = (N_SHARD + CB - 1) // CB     # 62
N_PAD = CB * NBLK                   # 63488
CHUNK = 512
CHUNKS = N_PAD // CHUNK             # 124
WMAP = 32               # map entries per chunk (window = 16 cols, stride 32)
WIN = CHUNK // WMAP     # 16
MAPN = CHUNKS * WMAP                # 3968 map entries per query block
REG_CHUNKS = 32         # chunks per top-8 region (<= 1024 map entries)
NREG = (CHUNKS + REG_CHUNKS - 1) // REG_CHUNKS   # 4
EPS_TIE = 2.0 ** -13

K = 32
TEMP = 0.07
NUM_CLASSES = 1000
EPS = 1e-12
W_SEL = 160             # windows kept per query by the host value prefilter

FP8 = mybir.dt.float8e4
NP_FP8 = mybir.dt.np(FP8)

# unit u = 2*block + qb; 'A': scalar-engine activation copy PSUM->SBUF bf16,
# 'V': DVE tensor_copy extract (gpsimd cannot read PSUM on TRN2).
# DVE runs the fold tree for all units.
def _sched(nunits):
    return ['A' for _ in range(nunits)]


_NC_CACHE = None


def build_nc(nblk=NBLK):
    n_pad = nblk * CB
    chunks = n_pad // CHUNK
    nreg = (chunks + REG_CHUNKS - 1) // REG_CHUNKS
    sched = _sched(nblk * QB)

    nc = bacc.Bacc("TRN2", target_bir_lowering=False, debug=False)
    q_dram = nc.dram_tensor("qT8", [P, 4, NQ], FP8, kind="ExternalInput").ap()
    f_dram = nc.dram_tensor("fT8", [P, 4, n_pad], FP8,
                            kind="ExternalInput").ap()
    iota_dram = nc.dram_tensor("iota", [P, REG_CHUNKS * WMAP],
                               mybir.dt.float32, kind="ExternalInput").ap()
    vals_dram = nc.dram_tensor("cand_vals", [QB, P, nreg * 8],
                               mybir.dt.float32, kind="ExternalOutput").ap()

    MAX = mybir.AluOpType.max

    with TileContext(nc) as tc:
        with (
            tc.tile_pool(name="qpool", bufs=1) as qpool,
            tc.tile_pool(name="fpool", bufs=4) as fpool,
            tc.tile_pool(name="spool", bufs=6) as spool,
            tc.tile_pool(name="tpool", bufs=4) as tpool,
            tc.tile_pool(name="mpool", bufs=1) as mpool,
            tc.tile_pool(name="rpool", bufs=2) as rpool,
            tc.tile_pool(name="psum", bufs=2, space="PSUM") as psum_pool,
        ):
            qt = qpool.tile([P, 4, NQ], FP8)
            nc.sync.dma_start(qt, q_dram)
            iota = qpool.tile([P, REG_CHUNKS * WMAP], mybir.dt.float32)

            maps = [mpool.tile([P, chunks, WMAP], mybir.dt.bfloat16,
                               name=f"map{qb}") for qb in range(QB)]
            vout = [mpool.tile([P, nreg * 8], mybir.dt.float32,
                               name=f"vout{qb}") for qb in range(QB)]

            for b in range(nblk):
                ft = fpool.tile([P, 4, CB], FP8, tag="ft")
                nc.sync.dma_start(ft, f_dram[:, :, b * CB:(b + 1) * CB])
                if b == 0:
                    # iota is only needed by the first region reduction
                    # (block 15); keep it off the critical startup path
                    nc.sync.dma_start(iota, iota_dram)

                # one 4-bank PSUM tile per block, qb-major: [qb, c, :]
                pt = psum_pool.tile([P, 4, CHUNK], mybir.dt.float32,
                                    name=f"pt_{b}", tag="pt")
                # weight reuse: same lhsT serves both 512-col chunks
                for kc in range(2):
                    for qb in range(QB):
                        lhsT = qt[:, 2 * kc:2 * kc + 2, qb * P:(qb + 1) * P]
                        for c in range(2):
                            nc.tensor.matmul(
                                pt[:, 2 * qb + c, :],
                                lhsT=lhsT,
                                rhs=ft[:, 2 * kc:2 * kc + 2,
                                       c * CHUNK:(c + 1) * CHUNK],
                                start=(kc == 0), stop=(kc == 1),
                                perf_mode=mybir.MatmulPerfMode.DoubleRow)

                # extract all 4 chunks in one scalar-engine pass (measured:
                # any DVE share of extraction delays folds + PSUM release)
                s = spool.tile([P, 4, CHUNK], mybir.dt.bfloat16, tag="s")
                nc.scalar.activation(
                    s, pt, func=mybir.ActivationFunctionType.Copy)
                t1 = tpool.tile([P, 4, 256], mybir.dt.bfloat16, tag="t1")
                nc.vector.tensor_tensor(
                    out=t1, in0=s[:, :, 0:256], in1=s[:, :, 256:512], op=MAX)
                t2 = tpool.tile([P, 4, 128], mybir.dt.bfloat16, tag="t2")
                nc.vector.tensor_tensor(
                    out=t2, in0=t1[:, :, 0:128], in1=t1[:, :, 128:256], op=MAX)
                t3 = tpool.tile([P, 4, 64], mybir.dt.bfloat16, tag="t3")
                nc.vector.tensor_tensor(
                    out=t3, in0=t2[:, :, 0:64], in1=t2[:, :, 64:128], op=MAX)
                for qb in range(QB):
                    nc.vector.tensor_tensor(
                        out=maps[qb][:, 2 * b:2 * b + 2, :],
                        in0=t3[:, 2 * qb:2 * qb + 2, 0:32],
                        in1=t3[:, 2 * qb:2 * qb + 2, 32:64], op=MAX)

                # emit region reductions as soon as their chunks are done
                for qb in range(QB):
                    for r in range(nreg):
                        lo = r * REG_CHUNKS
                        hi = min(lo + REG_CHUNKS, chunks)
                        if hi != 2 * (b + 1):
                            continue
                        cnt = (hi - lo) * WMAP
                        mp = rpool.tile([P, REG_CHUNKS * WMAP],
                                        mybir.dt.float32, tag="mp")
                        nc.vector.scalar_tensor_tensor(
                            out=mp[:, 0:cnt], in0=maps[qb][:, lo:hi, :],
                            scalar=1.0, in1=iota[:, 0:cnt],
                            op0=mybir.AluOpType.mult,
                            op1=mybir.AluOpType.add)
                        v8 = vout[qb][:, r * 8:(r + 1) * 8]
                        nc.vector.max(out=v8, in_=mp[:, 0:cnt])

            for qb in range(QB):
                nc.sync.dma_start(vals_dram[qb], vout[qb])
    nc.compile()
    return nc


def _get_nc():
    global _NC_CACHE
    if _NC_CACHE is None:
        _NC_CACHE = build_nc()
    return _NC_CACHE


def _prep_in_maps(queries, f):
    # queries -> [128, 4, 256] fp8: q8[p, s, m] = q[m, 128*s + p]
    qT = np.ascontiguousarray(queries.T)                     # [512, 256]
    q8 = np.ascontiguousarray(
        qT.reshape(4, P, NQ).transpose(1, 0, 2)).astype(NP_FP8)
    iota = np.broadcast_to(
        (np.arange(REG_CHUNKS * WMAP, dtype=np.float32) *
         np.float32(EPS_TIE))[None, :], (P, REG_CHUNKS * WMAP))
    iota = np.ascontiguousarray(iota)

    f8_full = f.astype(NP_FP8)                               # [500000, 512]
    in_maps = []
    for c in range(NCORES):
        shard = f8_full[c * N_SHARD:(c + 1) * N_SHARD]       # [62500, 512]
        fT = shard.T.reshape(4, P, N_SHARD).transpose(1, 0, 2)  # [128,4,N]
        f8 = np.zeros((P, 4, N_PAD), dtype=NP_FP8)
        f8[:, :, :N_SHARD] = fT
        in_maps.append({"qT8": q8, "fT8": f8, "iota": iota})
    return in_maps


def run_device(queries, f, trace=False):
    """Returns (wval, widx, res): per-core window values / global window ids.

    wval/widx: [NCORES, NQ, NREG*8]  (window id = map index in [0, MAPN)).
    The region-local index is decoded from the value's iota*2^-13 payload
    (exact whenever |value| >= 32; smaller values are never competitive)."""
    in_maps = _prep_in_maps(queries, f)
    nc = _get_nc()
    res = bass_utils.run_bass_kernel_spmd(
        nc, in_maps, core_ids=list(range(NCORES)), trace=trace)
    wval = np.stack([np.asarray(res.results[c]["cand_vals"],
                                dtype=np.float32).reshape(NQ, NREG * 8)
                     for c in range(NCORES)])
    b16 = wval.astype(ml_dtypes.bfloat16)
    base = b16.astype(np.float32)
    b_dn = np.nextafter(b16, ml_dtypes.bfloat16(-3e38)).astype(np.float32)
    base = np.where(base > wval, b_dn, base)
    pos = np.rint((wval - base) * np.float32(1.0 / EPS_TIE)).astype(np.int64)
    np.clip(pos, 0, REG_CHUNKS * WMAP - 1, out=pos)
    reg_of = np.arange(NREG * 8) // 8                        # [NREG*8]
    widx = reg_of[None, None, :] * (REG_CHUNKS * WMAP) + pos
    np.clip(widx, 0, MAPN - 1, out=widx)
    return wval, widx, res


def knn_from_windows(queries, f, labels, wval, widx):
    nq = queries.shape[0]
    nwin = NCORES * NREG * 8                                 # 256
    # flatten to [nq, nwin] with core-global window ids
    val = wval.transpose(1, 0, 2).reshape(nq, nwin)
    gwin = (widx + (np.arange(NCORES) * MAPN)[:, None, None]) \
        .transpose(1, 0, 2).reshape(nq, nwin)

    w = min(W_SEL, nwin)
    part = np.argpartition(-val, w - 1, axis=1)[:, :w]
    sel = np.take_along_axis(gwin, part, axis=1)             # [nq, w]

    # expand windows to columns: window j (within core) = chunk j>>5,
    # residue j&31 -> cols chunk*512 + residue + 32k, k=0..15
    core = sel // MAPN
    j = sel % MAPN
    ch = j >> 5
    rs = j & 31
    cols = (ch[:, :, None] * CHUNK + rs[:, :, None]
            + 32 * np.arange(WIN)[None, None, :])            # [nq, w, 16]
    rows = core[:, :, None] * N_SHARD + cols
    valid = cols < N_SHARD
    rows = np.where(valid, rows, 0).reshape(nq, -1)          # [nq, w*16]
    valid = valid.reshape(nq, -1)

    # exact fp32 rescore with the reference normalization
    qn = queries.astype(np.float32)
    qn /= np.maximum(np.linalg.norm(qn, axis=1, keepdims=True), EPS)
    rows_f = f[rows.reshape(-1)].reshape(nq, rows.shape[1], D)
    rows_f = rows_f / np.maximum(
        np.linalg.norm(rows_f, axis=2, keepdims=True), EPS)
    sims = np.einsum('qtd,qd->qt', rows_f, qn, dtype=np.float32)
    sims = np.where(valid, sims, -np.inf)

    # top-32, ties broken by lower row index (jax.lax.top_k semantics)
    by_idx = np.argsort(rows, axis=1, kind='stable')
    sims_s = np.take_along_axis(sims, by_idx, axis=1)
    rows_s = np.take_along_axis(rows, by_idx, axis=1)
    order = np.argsort(-sims_s, axis=1, kind='stable')[:, :K]
    top_sims = np.take_along_axis(sims_s, order, axis=1)
    top_idx = np.take_along_axis(rows_s, order, axis=1)

    wts = np.exp(top_sims.astype(np.float32) / np.float32(TEMP))
    lab = labels[top_idx]
    out = np.zeros((nq, NUM_CLASSES), dtype=np.float32)
    np.add.at(out, (np.arange(nq)[:, None], lab), wts)
    return out


def kernel(queries, train_features, train_labels):
    queries = np.asarray(queries, dtype=np.float32)
    f = np.asarray(train_features, dtype=np.float32)
    labels = np.asarray(train_labels)
    wval, widx, _ = run_device(queries, f)
    return knn_from_windows(queries, f, labels, wval, widx)


# revision 28
# speedup vs baseline: 1.1188x; 1.0260x over previous
"""KNN retrieval kernel for Trainium2 (8 NeuronCores, SPMD).

Pipeline (per core, datastore sharded N=500000 -> 8 x 62500):
  host:   shard + transpose features to [128, 4, N_loc] fp8-e4m3 (contraction
          subtile layout for DoubleRow matmuls), queries to [128, 4, 256] fp8.
  device: sims = q @ f in fp8 DoubleRow mode (K=256/pass, fp32 PSUM).
          Scalar engine (or DVE directly from PSUM) extracts each [128, 2x512]
          PSUM unit; DVE folds each 512-col chunk 16x by a pairwise-max tree
          (bf16, 2x perf mode) -> window-max map [128, 124*32] per query block
          (window = 16 cols with stride 32 inside its chunk).
          Per region of <=32 chunks: add iota*2^-13 (unique tie-break) ->
          fp32 map', vector.max top-8 + vector.max_index -> 8 window ids.
  host:   per query 256 candidate windows (8 cores x 32), keep top-160 by
          map value, expand to 16 cols each, exact fp32 rescore with true
          L2 normalization, top-32 with reference tie-break, W = exp(s/T),
          one-hot label aggregation.

Recall: a true top-32 neighbor is missed only if its window-max loses its
region's top-8 (margin ~5 sigma of fp8 noise) or the host top-160 filter
(~5 sigma). Window members are all rescored, so in-window collisions are
harmless.
"""

import numpy as np
import ml_dtypes

import concourse.bass as bass
import concourse.mybir as mybir
from concourse import bacc
from concourse.tile import TileContext
from concourse import bass_utils

P = 128
D = 512                 # feature dim = 4 K-subtiles of 128
NQ = 256                # queries (2 partition blocks)
QB = NQ // P            # 2
NCORES = 8
N_TOTAL = 500000
N_SHARD = N_TOTAL // NCORES    # 62500
CB = 1024               # columns per block (= 2 PSUM banks per query block)
NBLK = (N_SHARD + CB - 1) // CB     # 62
N_PAD = CB * NBLK                   # 63488
CHUNK = 512
CHUNKS = N_PAD // CHUNK             # 124
WMAP = 32               # map entries per chunk (window = 16 cols, stride 32)
WIN = CHUNK // WMAP     # 16
MAPN = CHUNKS * WMAP                # 3968 map entries per query block
REG_CHUNKS = 32         # max chunks per top-8 region (<= 1024 map entries)
# region boundaries in chunks; small last region keeps the drain short
REGIONS = [(0, 32), (32, 64), (64, 96), (96, 116), (116, 124)]
NREG = len(REGIONS)     # 5
EPS_TIE = 2.0 ** -13

K = 32
TEMP = 0.07
NUM_CLASSES = 1000
EPS = 1e-12
W_SEL = 160             # windows kept per query by the host value prefilter

FP8 = mybir.dt.float8e4
NP_FP8 = mybir.dt.np(FP8)

# unit u = 2*block + qb; 'A': scalar-engine activation copy PSUM->SBUF bf16,
# 'V': DVE tensor_copy extract (gpsimd cannot read PSUM on TRN2).
# DVE runs the fold tree for all units.
def _sched(nunits):
    return ['A' for _ in range(nunits)]


_NC_CACHE = None


def build_nc(nblk=NBLK):
    n_pad = nblk * CB
    chunks = n_pad // CHUNK
    if nblk == NBLK:
        regions = REGIONS
    else:
        regions = [(lo, min(lo + REG_CHUNKS, chunks))
                   for lo in range(0, chunks, REG_CHUNKS)]
    nreg = len(regions)

    nc = bacc.Bacc("TRN2", target_bir_lowering=False, debug=False)
    q_dram = nc.dram_tensor("qT8", [P, 4, NQ], FP8, kind="ExternalInput").ap()
    f_dram = nc.dram_tensor("fT8", [P, 4, n_pad], FP8,
                            kind="ExternalInput").ap()
    iota_dram = nc.dram_tensor("iota", [P, REG_CHUNKS * WMAP],
                               mybir.dt.float32, kind="ExternalInput").ap()
    vals_dram = nc.dram_tensor("cand_vals", [QB, P, nreg * 8],
                               mybir.dt.float32, kind="ExternalOutput").ap()

    MAX = mybir.AluOpType.max

    with TileContext(nc) as tc:
        with (
            tc.tile_pool(name="qpool", bufs=1) as qpool,
            tc.tile_pool(name="fpool", bufs=4) as fpool,
            tc.tile_pool(name="spool", bufs=6) as spool,
            tc.tile_pool(name="tpool", bufs=4) as tpool,
            tc.tile_pool(name="mpool", bufs=1) as mpool,
            tc.tile_pool(name="rpool", bufs=2) as rpool,
            tc.tile_pool(name="psum", bufs=2, space="PSUM") as psum_pool,
        ):
            qt = qpool.tile([P, 4, NQ], FP8)
            nc.sync.dma_start(qt, q_dram)
            iota = qpool.tile([P, REG_CHUNKS * WMAP], mybir.dt.float32)

            maps = [mpool.tile([P, chunks, WMAP], mybir.dt.bfloat16,
                               name=f"map{qb}") for qb in range(QB)]
            vout = [mpool.tile([P, nreg * 8], mybir.dt.float32,
                               name=f"vout{qb}") for qb in range(QB)]

            for b in range(nblk):
                # split the feature DMA by kc-half so kc=0 matmuls start
                # after 256KB instead of gating on the whole 512KB block
                fts = []
                for kc in range(2):
                    fth = fpool.tile([P, 2, CB], FP8, tag=f"ft{kc}")
                    nc.sync.dma_start(
                        fth, f_dram[:, 2 * kc:2 * kc + 2,
                                    b * CB:(b + 1) * CB])
                    fts.append(fth)
                if b == 0:
                    # iota is only needed by the first region reduction
                    # (block 15); keep it off the critical startup path
                    nc.sync.dma_start(iota, iota_dram)

                # one 4-bank PSUM tile per block, qb-major: [qb, c, :]
                pt = psum_pool.tile([P, 4, CHUNK], mybir.dt.float32,
                                    name=f"pt_{b}", tag="pt")
                # weight reuse: same lhsT serves both 512-col chunks
                for kc in range(2):
                    for qb in range(QB):
                        lhsT = qt[:, 2 * kc:2 * kc + 2, qb * P:(qb + 1) * P]
                        for c in range(2):
                            nc.tensor.matmul(
                                pt[:, 2 * qb + c, :],
                                lhsT=lhsT,
                                rhs=fts[kc][:, :,
                                            c * CHUNK:(c + 1) * CHUNK],
                                start=(kc == 0), stop=(kc == 1),
                                perf_mode=mybir.MatmulPerfMode.DoubleRow)

                # extract all 4 chunks in one scalar-engine pass (measured:
                # any DVE share of extraction delays folds + PSUM release)
                s = spool.tile([P, 4, CHUNK], mybir.dt.bfloat16, tag="s")
                nc.scalar.activation(
                    s, pt, func=mybir.ActivationFunctionType.Copy)
                t1 = tpool.tile([P, 4, 256], mybir.dt.bfloat16, tag="t1")
                nc.vector.tensor_tensor(
                    out=t1, in0=s[:, :, 0:256], in1=s[:, :, 256:512], op=MAX)
                t2 = tpool.tile([P, 4, 128], mybir.dt.bfloat16, tag="t2")
                nc.vector.tensor_tensor(
                    out=t2, in0=t1[:, :, 0:128], in1=t1[:, :, 128:256], op=MAX)
                t3 = tpool.tile([P, 4, 64], mybir.dt.bfloat16, tag="t3")
                nc.vector.tensor_tensor(
                    out=t3, in0=t2[:, :, 0:64], in1=t2[:, :, 64:128], op=MAX)
                for qb in range(QB):
                    nc.vector.tensor_tensor(
                        out=maps[qb][:, 2 * b:2 * b + 2, :],
                        in0=t3[:, 2 * qb:2 * qb + 2, 0:32],
                        in1=t3[:, 2 * qb:2 * qb + 2, 32:64], op=MAX)

                # emit region reductions as soon as their chunks are done
                for qb in range(QB):
                    for r, (lo, hi) in enumerate(regions):
                        if hi != 2 * (b + 1):
                            continue
                        cnt = (hi - lo) * WMAP
                        mp = rpool.tile([P, REG_CHUNKS * WMAP],
                                        mybir.dt.float32, tag="mp")
                        nc.vector.scalar_tensor_tensor(
                            out=mp[:, 0:cnt], in0=maps[qb][:, lo:hi, :],
                            scalar=1.0, in1=iota[:, 0:cnt],
                            op0=mybir.AluOpType.mult,
                            op1=mybir.AluOpType.add)
                        v8 = vout[qb][:, r * 8:(r + 1) * 8]
                        nc.vector.max(out=v8, in_=mp[:, 0:cnt])

            for qb in range(QB):
                nc.sync.dma_start(vals_dram[qb], vout[qb])
    nc.compile()
    return nc


def _get_nc():
    global _NC_CACHE
    if _NC_CACHE is None:
        _NC_CACHE = build_nc()
    return _NC_CACHE


def _prep_in_maps(queries, f):
    # queries -> [128, 4, 256] fp8: q8[p, s, m] = q[m, 128*s + p]
    qT = np.ascontiguousarray(queries.T)                     # [512, 256]
    q8 = np.ascontiguousarray(
        qT.reshape(4, P, NQ).transpose(1, 0, 2)).astype(NP_FP8)
    iota = np.broadcast_to(
        (np.arange(REG_CHUNKS * WMAP, dtype=np.float32) *
         np.float32(EPS_TIE))[None, :], (P, REG_CHUNKS * WMAP))
    iota = np.ascontiguousarray(iota)

    f8_full = f.astype(NP_FP8)                               # [500000, 512]
    in_maps = []
    for c in range(NCORES):
        shard = f8_full[c * N_SHARD:(c + 1) * N_SHARD]       # [62500, 512]
        fT = shard.T.reshape(4, P, N_SHARD).transpose(1, 0, 2)  # [128,4,N]
        f8 = np.zeros((P, 4, N_PAD), dtype=NP_FP8)
        f8[:, :, :N_SHARD] = fT
        in_maps.append({"qT8": q8, "fT8": f8, "iota": iota})
    return in_maps


def run_device(queries, f, trace=False):
    """Returns (wval, widx, res): per-core window values / global window ids.

    wval/widx: [NCORES, NQ, NREG*8]  (window id = map index in [0, MAPN)).
    The region-local index is decoded from the value's iota*2^-13 payload
    (exact whenever |value| >= 32; smaller values are never competitive)."""
    in_maps = _prep_in_maps(queries, f)
    nc = _get_nc()
    res = bass_utils.run_bass_kernel_spmd(
        nc, in_maps, core_ids=list(range(NCORES)), trace=trace)
    wval = np.stack([np.asarray(res.results[c]["cand_vals"],
                                dtype=np.float32).reshape(NQ, NREG * 8)
                     for c in range(NCORES)])
    b16 = wval.astype(ml_dtypes.bfloat16)
    base = b16.astype(np.float32)
    b_dn = np.nextafter(b16, ml_dtypes.bfloat16(-3e38)).astype(np.float32)
    base = np.where(base > wval, b_dn, base)
    pos = np.rint((wval - base) * np.float32(1.0 / EPS_TIE)).astype(np.int64)
    sz_of = np.repeat([(hi - lo) * WMAP for lo, hi in REGIONS], 8)
    np.clip(pos, 0, sz_of[None, None, :] - 1, out=pos)
    lo_of = np.repeat([lo * WMAP for lo, hi in REGIONS], 8)  # [NREG*8]
    widx = lo_of[None, None, :] + pos
    np.clip(widx, 0, MAPN - 1, out=widx)
    return wval, widx, res


def knn_from_windows(queries, f, labels, wval, widx):
    nq = queries.shape[0]
    nwin = NCORES * NREG * 8                                 # 256
    # flatten to [nq, nwin] with core-global window ids
    val = wval.transpose(1, 0, 2).reshape(nq, nwin)
    gwin = (widx + (np.arange(NCORES) * MAPN)[:, None, None]) \
        .transpose(1, 0, 2).reshape(nq, nwin)

    w = min(W_SEL, nwin)
    part = np.argpartition(-val, w - 1, axis=1)[:, :w]
    sel = np.take_along_axis(gwin, part, axis=1)             # [nq, w]

    # expand windows to columns: window j (within core) = chunk j>>5,
    # residue j&31 -> cols chunk*512 + residue + 32k, k=0..15
    core = sel // MAPN
    j = sel % MAPN
    ch = j >> 5
    rs = j & 31
    cols = (ch[:, :, None] * CHUNK + rs[:, :, None]
            + 32 * np.arange(WIN)[None, None, :])            # [nq, w, 16]
    rows = core[:, :, None] * N_SHARD + cols
    valid = cols < N_SHARD
    rows = np.where(valid, rows, 0).reshape(nq, -1)          # [nq, w*16]
    valid = valid.reshape(nq, -1)

    # exact fp32 rescore with the reference normalization
    qn = queries.astype(np.float32)
    qn /= np.maximum(np.linalg.norm(qn, axis=1, keepdims=True), EPS)
    rows_f = f[rows.reshape(-1)].reshape(nq, rows.shape[1], D)
    rows_f = rows_f / np.maximum(
        np.linalg.norm(rows_f, axis=2, keepdims=True), EPS)
    sims = np.einsum('qtd,qd->qt', rows_f, qn, dtype=np.float32)
    sims = np.where(valid, sims, -np.inf)

    # top-32, ties broken by lower row index (jax.lax.top_k semantics)
    by_idx = np.argsort(rows, axis=1, kind='stable')
    sims_s = np.take_along_axis(sims, by_idx, axis=1)
    rows_s = np.take_along_axis(rows, by_idx, axis=1)
    order = np.argsort(-sims_s, axis=1, kind='stable')[:, :K]
    top_sims = np.take_along_axis(sims_s, order, axis=1)
    top_idx = np.take_along_axis(rows_s, order, axis=1)

    wts = np.exp(top_sims.astype(np.float32) / np.float32(TEMP))
    lab = labels[top_idx]
    out = np.zeros((nq, NUM_CLASSES), dtype=np.float32)
    np.add.at(out, (np.arange(nq)[:, None], lab), wts)
    return out


def kernel(queries, train_features, train_labels):
    queries = np.asarray(queries, dtype=np.float32)
    f = np.asarray(train_features, dtype=np.float32)
    labels = np.asarray(train_labels)
    wval, widx, _ = run_device(queries, f)
    return knn_from_windows(queries, f, labels, wval, widx)


# revision 32
# speedup vs baseline: 1.1415x; 1.0203x over previous
"""KNN retrieval kernel for Trainium2 (8 NeuronCores, SPMD).

Pipeline (per core, datastore sharded N=500000 -> 8 x 62500):
  host:   shard + transpose features to [128, 4, N_loc] fp8-e4m3 (contraction
          subtile layout for DoubleRow matmuls), queries to [128, 4, 256] fp8.
  device: sims = q @ f in fp8 DoubleRow mode (K=256/pass, fp32 PSUM).
          Scalar engine (or DVE directly from PSUM) extracts each [128, 2x512]
          PSUM unit; DVE folds each 512-col chunk 16x by a pairwise-max tree
          (bf16, 2x perf mode) -> window-max map [128, 124*32] per query block
          (window = 16 cols with stride 32 inside its chunk).
          Per region of <=32 chunks: add iota*2^-13 (unique tie-break) ->
          fp32 map', vector.max top-8 + vector.max_index -> 8 window ids.
  host:   per query 256 candidate windows (8 cores x 32), keep top-160 by
          map value, expand to 16 cols each, exact fp32 rescore with true
          L2 normalization, top-32 with reference tie-break, W = exp(s/T),
          one-hot label aggregation.

Recall: a true top-32 neighbor is missed only if its window-max loses its
region's top-8 (margin ~5 sigma of fp8 noise) or the host top-160 filter
(~5 sigma). Window members are all rescored, so in-window collisions are
harmless.
"""

import numpy as np
import ml_dtypes

import concourse.bass as bass
import concourse.mybir as mybir
from concourse import bacc
from concourse.tile import TileContext
from concourse import bass_utils

P = 128
D = 512                 # feature dim = 4 K-subtiles of 128
NQ = 256                # queries (2 partition blocks)
QB = NQ // P            # 2
NCORES = 8
N_TOTAL = 500000
N_SHARD = N_TOTAL // NCORES    # 62500
CB = 1024               # columns per block (= 2 PSUM banks per query block)
NBLK = N_SHARD // CB                # 61 full blocks; the 36-col tail of each
N_PAD = CB * NBLK                   # shard (62464..62500) is rescored on the
CHUNK = 512                         # host exactly instead of padding a block
CHUNKS = N_PAD // CHUNK             # 122
WMAP = 32               # map entries per chunk (window = 16 cols, stride 32)
WIN = CHUNK // WMAP     # 16
MAPN = CHUNKS * WMAP                # 3904 map entries per query block
REG_CHUNKS = 32         # max chunks per top-8 region (<= 1024 map entries)
# region boundaries in chunks; small last region keeps the drain short
REGIONS = [(0, 32), (32, 64), (64, 96), (96, 114), (114, 122)]
NREG = len(REGIONS)     # 5
EPS_TIE = 2.0 ** -13

K = 32
TEMP = 0.07
NUM_CLASSES = 1000
EPS = 1e-12
W_SEL = 160             # windows kept per query by the host value prefilter

FP8 = mybir.dt.float8e4
NP_FP8 = mybir.dt.np(FP8)

# unit u = 2*block + qb; 'A': scalar-engine activation copy PSUM->SBUF bf16,
# 'V': DVE tensor_copy extract (gpsimd cannot read PSUM on TRN2).
# DVE runs the fold tree for all units.
def _sched(nunits):
    return ['A' for _ in range(nunits)]


_NC_CACHE = None


def build_nc(nblk=NBLK):
    n_pad = nblk * CB
    chunks = n_pad // CHUNK
    if nblk == NBLK:
        regions = REGIONS
    else:
        regions = [(lo, min(lo + REG_CHUNKS, chunks))
                   for lo in range(0, chunks, REG_CHUNKS)]
    nreg = len(regions)

    nc = bacc.Bacc("TRN2", target_bir_lowering=False, debug=False)
    q_dram = nc.dram_tensor("qT8", [P, 4, NQ], FP8, kind="ExternalInput").ap()
    f_dram = nc.dram_tensor("fT8", [P, 4, n_pad], FP8,
                            kind="ExternalInput").ap()
    iota_dram = nc.dram_tensor("iota", [P, REG_CHUNKS * WMAP],
                               mybir.dt.float32, kind="ExternalInput").ap()
    vals_dram = nc.dram_tensor("cand_vals", [QB, P, nreg * 8],
                               mybir.dt.float32, kind="ExternalOutput").ap()

    MAX = mybir.AluOpType.max

    with TileContext(nc) as tc:
        with (
            tc.tile_pool(name="qpool", bufs=1) as qpool,
            tc.tile_pool(name="fpool", bufs=4) as fpool,
            tc.tile_pool(name="spool", bufs=6) as spool,
            tc.tile_pool(name="tpool", bufs=4) as tpool,
            tc.tile_pool(name="mpool", bufs=1) as mpool,
            tc.tile_pool(name="rpool", bufs=2) as rpool,
            tc.tile_pool(name="psum", bufs=2, space="PSUM") as psum_pool,
        ):
            qt = qpool.tile([P, 4, NQ], FP8)
            nc.sync.dma_start(qt, q_dram)
            iota = qpool.tile([P, REG_CHUNKS * WMAP], mybir.dt.float32)

            maps = [mpool.tile([P, chunks, WMAP], mybir.dt.bfloat16,
                               name=f"map{qb}") for qb in range(QB)]
            vout = [mpool.tile([P, nreg * 8], mybir.dt.float32,
                               name=f"vout{qb}") for qb in range(QB)]

            for b in range(nblk):
                # split the feature DMA by kc-half so kc=0 matmuls start
                # after 256KB instead of gating on the whole 512KB block
                fts = []
                for kc in range(2):
                    fth = fpool.tile([P, 2, CB], FP8, tag=f"ft{kc}")
                    nc.sync.dma_start(
                        fth, f_dram[:, 2 * kc:2 * kc + 2,
                                    b * CB:(b + 1) * CB])
                    fts.append(fth)
                if b == 0:
                    # iota is only needed by the first region reduction
                    # (block 15); keep it off the critical startup path
                    nc.sync.dma_start(iota, iota_dram)

                # one 4-bank PSUM tile per block, qb-major: [qb, c, :]
                pt = psum_pool.tile([P, 4, CHUNK], mybir.dt.float32,
                                    name=f"pt_{b}", tag="pt")
                # weight reuse: same lhsT serves both 512-col chunks
                for kc in range(2):
                    for qb in range(QB):
                        lhsT = qt[:, 2 * kc:2 * kc + 2, qb * P:(qb + 1) * P]
                        for c in range(2):
                            nc.tensor.matmul(
                                pt[:, 2 * qb + c, :],
                                lhsT=lhsT,
                                rhs=fts[kc][:, :,
                                            c * CHUNK:(c + 1) * CHUNK],
                                start=(kc == 0), stop=(kc == 1),
                                perf_mode=mybir.MatmulPerfMode.DoubleRow)

                # extract all 4 chunks in one scalar-engine pass (measured:
                # any DVE share of extraction delays folds + PSUM release)
                s = spool.tile([P, 4, CHUNK], mybir.dt.bfloat16, tag="s")
                nc.scalar.activation(
                    s, pt, func=mybir.ActivationFunctionType.Copy)
                t1 = tpool.tile([P, 4, 256], mybir.dt.bfloat16, tag="t1")
                nc.vector.tensor_tensor(
                    out=t1, in0=s[:, :, 0:256], in1=s[:, :, 256:512], op=MAX)
                t2 = tpool.tile([P, 4, 128], mybir.dt.bfloat16, tag="t2")
                nc.vector.tensor_tensor(
                    out=t2, in0=t1[:, :, 0:128], in1=t1[:, :, 128:256], op=MAX)
                t3 = tpool.tile([P, 4, 64], mybir.dt.bfloat16, tag="t3")
                nc.vector.tensor_tensor(
                    out=t3, in0=t2[:, :, 0:64], in1=t2[:, :, 64:128], op=MAX)
                for qb in range(QB):
                    nc.vector.tensor_tensor(
                        out=maps[qb][:, 2 * b:2 * b + 2, :],
                        in0=t3[:, 2 * qb:2 * qb + 2, 0:32],
                        in1=t3[:, 2 * qb:2 * qb + 2, 32:64], op=MAX)

                # emit region reductions as soon as their chunks are done
                for qb in range(QB):
                    for r, (lo, hi) in enumerate(regions):
                        if hi != 2 * (b + 1):
                            continue
                        cnt = (hi - lo) * WMAP
                        mp = rpool.tile([P, REG_CHUNKS * WMAP],
                                        mybir.dt.float32, tag="mp")
                        nc.vector.scalar_tensor_tensor(
                            out=mp[:, 0:cnt], in0=maps[qb][:, lo:hi, :],
                            scalar=1.0, in1=iota[:, 0:cnt],
                            op0=mybir.AluOpType.mult,
                            op1=mybir.AluOpType.add)
                        v8 = vout[qb][:, r * 8:(r + 1) * 8]
                        nc.vector.max(out=v8, in_=mp[:, 0:cnt])
                        # stream each region's result out immediately so the
                        # final drain only waits on the last small region
                        nc.sync.dma_start(
                            vals_dram[qb][:, r * 8:(r + 1) * 8], v8)
    nc.compile()
    return nc


def _get_nc():
    global _NC_CACHE
    if _NC_CACHE is None:
        _NC_CACHE = build_nc()
    return _NC_CACHE


def _prep_in_maps(queries, f):
    # queries -> [128, 4, 256] fp8: q8[p, s, m] = q[m, 128*s + p]
    qT = np.ascontiguousarray(queries.T)                     # [512, 256]
    q8 = np.ascontiguousarray(
        qT.reshape(4, P, NQ).transpose(1, 0, 2)).astype(NP_FP8)
    iota = np.broadcast_to(
        (np.arange(REG_CHUNKS * WMAP, dtype=np.float32) *
         np.float32(EPS_TIE))[None, :], (P, REG_CHUNKS * WMAP))
    iota = np.ascontiguousarray(iota)

    f8_full = f.astype(NP_FP8)                               # [500000, 512]
    in_maps = []
    for c in range(NCORES):
        shard = f8_full[c * N_SHARD:(c + 1) * N_SHARD]       # [62500, 512]
        f8 = np.ascontiguousarray(
            shard.T[:, :N_PAD].reshape(4, P, N_PAD).transpose(1, 0, 2))
        in_maps.append({"qT8": q8, "fT8": f8, "iota": iota})
    return in_maps


def run_device(queries, f, trace=False):
    """Returns (wval, widx, res): per-core window values / global window ids.

    wval/widx: [NCORES, NQ, NREG*8]  (window id = map index in [0, MAPN)).
    The region-local index is decoded from the value's iota*2^-13 payload
    (exact whenever |value| >= 32; smaller values are never competitive)."""
    in_maps = _prep_in_maps(queries, f)
    nc = _get_nc()
    res = bass_utils.run_bass_kernel_spmd(
        nc, in_maps, core_ids=list(range(NCORES)), trace=trace)
    wval = np.stack([np.asarray(res.results[c]["cand_vals"],
                                dtype=np.float32).reshape(NQ, NREG * 8)
                     for c in range(NCORES)])
    b16 = wval.astype(ml_dtypes.bfloat16)
    base = b16.astype(np.float32)
    b_dn = np.nextafter(b16, ml_dtypes.bfloat16(-3e38)).astype(np.float32)
    base = np.where(base > wval, b_dn, base)
    pos = np.rint((wval - base) * np.float32(1.0 / EPS_TIE)).astype(np.int64)
    sz_of = np.repeat([(hi - lo) * WMAP for lo, hi in REGIONS], 8)
    np.clip(pos, 0, sz_of[None, None, :] - 1, out=pos)
    lo_of = np.repeat([lo * WMAP for lo, hi in REGIONS], 8)  # [NREG*8]
    widx = lo_of[None, None, :] + pos
    np.clip(widx, 0, MAPN - 1, out=widx)
    return wval, widx, res


def knn_from_windows(queries, f, labels, wval, widx):
    nq = queries.shape[0]
    nwin = NCORES * NREG * 8                                 # 256
    # flatten to [nq, nwin] with core-global window ids
    val = wval.transpose(1, 0, 2).reshape(nq, nwin)
    gwin = (widx + (np.arange(NCORES) * MAPN)[:, None, None]) \
        .transpose(1, 0, 2).reshape(nq, nwin)

    w = min(W_SEL, nwin)
    part = np.argpartition(-val, w - 1, axis=1)[:, :w]
    sel = np.take_along_axis(gwin, part, axis=1)             # [nq, w]

    # expand windows to columns: window j (within core) = chunk j>>5,
    # residue j&31 -> cols chunk*512 + residue + 32k, k=0..15
    core = sel // MAPN
    j = sel % MAPN
    ch = j >> 5
    rs = j & 31
    cols = (ch[:, :, None] * CHUNK + rs[:, :, None]
            + 32 * np.arange(WIN)[None, None, :])            # [nq, w, 16]
    rows = core[:, :, None] * N_SHARD + cols
    valid = cols < N_SHARD
    rows = np.where(valid, rows, 0).reshape(nq, -1)          # [nq, w*16]
    valid = valid.reshape(nq, -1)

    # the device never sees cols N_PAD..N_SHARD (36 per core); rescore them
    # for every query unconditionally
    tail = (np.arange(NCORES)[:, None] * N_SHARD
            + np.arange(N_PAD, N_SHARD)[None, :]).reshape(-1)   # [288]
    rows = np.concatenate(
        [rows, np.broadcast_to(tail[None, :], (nq, tail.size))], axis=1)
    valid = np.concatenate(
        [valid, np.ones((nq, tail.size), dtype=bool)], axis=1)

    # exact fp32 rescore with the reference normalization
    qn = queries.astype(np.float32)
    qn /= np.maximum(np.linalg.norm(qn, axis=1, keepdims=True), EPS)
    rows_f = f[rows.reshape(-1)].reshape(nq, rows.shape[1], D)
    rows_f = rows_f / np.maximum(
        np.linalg.norm(rows_f, axis=2, keepdims=True), EPS)
    sims = np.einsum('qtd,qd->qt', rows_f, qn, dtype=np.float32)
    sims = np.where(valid, sims, -np.inf)

    # top-32, ties broken by lower row index (jax.lax.top_k semantics)
    by_idx = np.argsort(rows, axis=1, kind='stable')
    sims_s = np.take_along_axis(sims, by_idx, axis=1)
    rows_s = np.take_along_axis(rows, by_idx, axis=1)
    order = np.argsort(-sims_s, axis=1, kind='stable')[:, :K]
    top_sims = np.take_along_axis(sims_s, order, axis=1)
    top_idx = np.take_along_axis(rows_s, order, axis=1)

    wts = np.exp(top_sims.astype(np.float32) / np.float32(TEMP))
    lab = labels[top_idx]
    out = np.zeros((nq, NUM_CLASSES), dtype=np.float32)
    np.add.at(out, (np.arange(nq)[:, None], lab), wts)
    return out


def kernel(queries, train_features, train_labels):
    queries = np.asarray(queries, dtype=np.float32)
    f = np.asarray(train_features, dtype=np.float32)
    labels = np.asarray(train_labels)
    wval, widx, _ = run_device(queries, f)
    return knn_from_windows(queries, f, labels, wval, widx)
